# revision 23
# baseline (speedup 1.0000x reference)
"""Complex AttentionPool2d on 8 trn2 NeuronCores, data-parallel over batch.

Contract: kernel(**inputs) takes the FULL inputs from setup_inputs() and
returns the FULL [32, 512] complex64 output.

v3 design: all matmuls in bf16 (tolerance is 2e-2), Karatsuba (3 real
matmuls) for the dominant k/v projections, and every small/serial piece of
math moved to the host:
  host: pos folded into the shipped x (x' = pixels + pos, pre-paired
        [pair, E, 2b, 256s]); x0 = mean(x)+pos0; q0 = (x0 Wq + b_q)/8;
        k0 = x0 Wk; v0 = x0 Wv; logit[s=0] = q0.k0; block-diag bd tiles
        from q0; v0 row-pairs; y-bias added to the final output on host.
  device (per core, 4 batches as 2 column-packed pairs):
        kT[f, (b,s)]   = Wk @ x'                 # Karatsuba, f-major
        v[(b,st)]      = x'^T @ Wv               # Karatsuba, s-major
        logits[8h,256] = bd^T @ kT per batch; col 0 from host
        w = softmax(re) + i softmax(im)          # exp straight from PSUM
        attn0 = w^T v (+ w0 x v0 row term)       # per batch [8, 512]
        y = attn0 @ (w_p w_out)^T                # via sel-extracted att^T

Engine rules honored: GPSIMD can't touch PSUM; vector ops read at most one
PSUM operand (stage via scalar-engine copies); compute engines can't write
at non-32-aligned partition offsets (assemble via DMA).

Math identities: k-bias dropped (softmax invariant); v-bias exits through
sum(w)=1 as a constant y-offset (host-added); q-bias folded into host q0.
"""
import contextlib
import numpy as np
import ml_dtypes

B, E, HW, S = 32, 512, 256, 257
NH, HD = 8, 64
OUT = 512
NCORES = 8
BPC = B // NCORES   # batches per core
NPAIR = BPC // 2    # column-packed batch pairs
BF16 = ml_dtypes.bfloat16

_cached = {}


def _build():
    import concourse.bacc as bacc
    import concourse.tile as tile
    import concourse.mybir as mybir

    F32 = mybir.dt.float32
    BF = mybir.dt.bfloat16
    ACTF = mybir.ActivationFunctionType

    nc = bacc.Bacc("TRN2", target_bir_lowering=False, debug=False)

    # ---- DRAM I/O ----
    F8 = mybir.dt.float8e4
    d_xr = nc.dram_tensor("xr", [NPAIR, E, 2, HW], BF, kind="ExternalInput")
    d_xi = nc.dram_tensor("xi", [NPAIR, E, 2, HW], BF, kind="ExternalInput")
    d_x8r = nc.dram_tensor("x8r", [NPAIR, 2, 128, 2, 512], F8, kind="ExternalInput")
    d_x8i = nc.dram_tensor("x8i", [NPAIR, 2, 128, 2, 512], F8, kind="ExternalInput")
    d_w8r = nc.dram_tensor("w8r", [2, 128, 2, 512], F8, kind="ExternalInput")
    d_w8i = nc.dram_tensor("w8i", [2, 128, 2, 512], F8, kind="ExternalInput")
    d_w8n = nc.dram_tensor("w8n", [2, 128, 2, 512], F8, kind="ExternalInput")
    d_wr = nc.dram_tensor("wr", [E, 2 * E], BF, kind="ExternalInput")
    d_wi = nc.dram_tensor("wi", [E, 2 * E], BF, kind="ExternalInput")
    d_ws = nc.dram_tensor("ws", [E, 2 * E], BF, kind="ExternalInput")
    d_wcr = nc.dram_tensor("wcr", [E, OUT], BF, kind="ExternalInput")
    d_wci = nc.dram_tensor("wci", [E, OUT], BF, kind="ExternalInput")
    d_bdr = nc.dram_tensor("bdr", [E, 32], BF, kind="ExternalInput")
    d_bdi = nc.dram_tensor("bdi", [E, 32], BF, kind="ExternalInput")
    d_bdin = nc.dram_tensor("bdin", [E, 32], BF, kind="ExternalInput")
    d_lg0 = nc.dram_tensor("lg0", [32, 2], F32, kind="ExternalInput")
    d_v02 = nc.dram_tensor("v02", [2, BPC, OUT], BF, kind="ExternalInput")
    d_v02s = nc.dram_tensor("v02s", [2, BPC, OUT], BF, kind="ExternalInput")
    d_id32 = nc.dram_tensor("id32", [32, 32], BF, kind="ExternalInput")
    d_mask = nc.dram_tensor("mask8", [NH, OUT], BF, kind="ExternalInput")
    d_sel = nc.dram_tensor("sel32", [32, BPC], BF, kind="ExternalInput")
    d_yr = nc.dram_tensor("yr", [BPC, OUT], F32, kind="ExternalOutput")
    d_yi = nc.dram_tensor("yi", [BPC, OUT], F32, kind="ExternalOutput")

    KS = slice(0, 512)       # k columns of the packed kv weight
    VS = slice(512, 1024)    # v columns

    with tile.TileContext(nc) as tc, \
         nc.allow_low_precision(reason="bf16 kernel; tolerance is 2e-2"):
        with tc.tile_pool(name="consts", bufs=1) as consts, \
             tc.tile_pool(name="keep", bufs=1) as keep:
            # ---- persistent weights / constants ----
            w_r = [consts.tile([128, 2 * E], BF, name=f"wr{e}") for e in range(4)]
            w_i = [consts.tile([128, 2 * E], BF, name=f"wi{e}") for e in range(4)]
            w_s = [consts.tile([128, 2 * E], BF, name=f"ws{e}") for e in range(4)]
            wcr = [consts.tile([128, OUT], BF, name=f"wcr{e}") for e in range(4)]
            wci = [consts.tile([128, OUT], BF, name=f"wci{e}") for e in range(4)]
            bd_r = [consts.tile([128, 32], BF, name=f"bdr{u}") for u in range(4)]
            bd_i = [consts.tile([128, 32], BF, name=f"bdi{u}") for u in range(4)]
            bd_in = [consts.tile([128, 32], BF, name=f"bdin{u}") for u in range(4)]
            lg0 = consts.tile([32, 2], F32)
            v02 = consts.tile([2, BPC, OUT], BF)
            v02s = consts.tile([2, BPC, OUT], BF)
            id32 = consts.tile([32, 32], BF)
            mask8 = consts.tile([NH, OUT], BF)
            sel32 = consts.tile([32, BPC], BF)

            x8 = {dt: [[consts.tile([128, 2, 512], F8, name=f"x8{dt}_{p}_{c}")
                        for c in range(2)] for p in range(NPAIR)]
                  for dt in "ri"}
            w8 = {dt: [consts.tile([128, 2, 512], F8, name=f"w8{dt}_{c}")
                       for c in range(2)] for dt in "rin"}

            # ---- persistent activations ----
            xr_t = [[keep.tile([128, 512], BF, name=f"xr{p}_{e}")
                     for e in range(4)] for p in range(NPAIR)]
            xi_t = [[keep.tile([128, 512], BF, name=f"xi{p}_{e}")
                     for e in range(4)] for p in range(NPAIR)]
            xs_t = [[keep.tile([128, 512], BF, name=f"xs{p}_{e}")
                     for e in range(4)] for p in range(NPAIR)]
            kTr = [[keep.tile([128, 512], BF, name=f"kTr{p}_{u}")
                    for u in range(4)] for p in range(NPAIR)]
            kTi = [[keep.tile([128, 512], BF, name=f"kTi{p}_{u}")
                    for u in range(4)] for p in range(NPAIR)]
            vr = [[keep.tile([128, OUT], BF, name=f"vr{b}_{s}")
                   for s in range(2)] for b in range(BPC)]
            vi = [[keep.tile([128, OUT], BF, name=f"vi{b}_{s}")
                   for s in range(2)] for b in range(BPC)]
            w_sm = keep.tile([32, 2, S], BF)        # softmax weights (re|im)
            wexp = [keep.tile([NH, 2, 256], BF, name=f"wexp{b}")
                    for b in range(BPC)]            # per-batch exp staging
            den8 = [keep.tile([NH, 2], F32, name=f"den8_{b}") for b in range(BPC)]
            den = keep.tile([32, 2], F32)           # exp row-sums of cols 1..256
            e0 = keep.tile([32, 2], F32)            # exp of the s=0 logit
            den2 = keep.tile([32, 2], F32)
            rs = keep.tile([32, 2], F32)
            wTr = [keep.tile([128, 32], BF, name=f"wTr{a}") for a in range(2)]
            wTi = [keep.tile([128, 32], BF, name=f"wTi{a}") for a in range(2)]
            wTin = [keep.tile([128, 32], BF, name=f"wTin{a}") for a in range(2)]
            wt0a = keep.tile([2, 32], BF)           # rows (w0r, -w0i)
            wt0b = keep.tile([2, 32], BF)           # rows (w0r, w0i)
            wt0n = keep.tile([2, 32], BF)
            hvm_r = keep.tile([32, OUT], BF)
            hvm_i = keep.tile([32, OUT], BF)
            hvm_rb = [keep.tile([NH, OUT], BF, name=f"hvr{b}") for b in range(BPC)]
            hvm_ib = [keep.tile([NH, OUT], BF, name=f"hvi{b}") for b in range(BPC)]
            att_r = [keep.tile([128, BPC], BF, name=f"atr{u}") for u in range(4)]
            att_i = [keep.tile([128, BPC], BF, name=f"ati{u}") for u in range(4)]
            att_in = [keep.tile([128, BPC], BF, name=f"atn{u}") for u in range(4)]
            s12 = [keep.tile([128, 512], F32, name=f"s12_{j}") for j in range(2)]
            c2s = [keep.tile([128, 512], F32, name=f"c2_{j}") for j in range(2)]
            c1s = [keep.tile([128, 512], F32, name=f"c1_{j}") for j in range(2)]
            y_r = keep.tile([BPC, OUT], F32)
            y_i = keep.tile([BPC, OUT], F32)

            # PE p-state warm-up: ~3us of dummy matmuls while DMAs land
            dummy = keep.tile([128, 512], BF)
            nc.vector.memset(dummy[:], 0.0)
            with tc.tile_pool(name="psW", bufs=1, space="PSUM") as psW:
                pw = psW.tile([128, 128], F32, tag="w", bufs=2)
                for j in range(14):
                    nc.tensor.matmul(pw[:], dummy[:, 0:128], dummy[:, 0:128],
                                     start=True, stop=True)

            # ---- DMA emission; first-needed bytes first ----
            # fp8 k-path: weights on sync/scalar, x8 on gpsimd
            for c in range(2):
                nc.sync.dma_start(out=w8["r"][c], in_=d_w8r.ap()[c])
                nc.scalar.dma_start(out=w8["i"][c], in_=d_w8i.ap()[c])
                nc.sync.dma_start(out=w8["n"][c], in_=d_w8n.ap()[c])
            for c in range(2):
                nc.gpsimd.dma_start(out=x8["r"][0][c], in_=d_x8r.ap()[0, c])
                nc.gpsimd.dma_start(out=x8["i"][0][c], in_=d_x8i.ap()[0, c])
            for e in range(4):
                sl = slice(e * 128, (e + 1) * 128)
                nc.gpsimd.dma_start(out=xr_t[0][e][:], in_=d_xr.ap()[0, sl, :, :])
            for c in range(2):
                nc.gpsimd.dma_start(out=x8["r"][1][c], in_=d_x8r.ap()[1, c])
                nc.gpsimd.dma_start(out=x8["i"][1][c], in_=d_x8i.ap()[1, c])
            for e in range(4):
                sl = slice(e * 128, (e + 1) * 128)
                nc.gpsimd.dma_start(out=xi_t[0][e][:], in_=d_xi.ap()[0, sl, :, :])
            for e in range(4):
                sl = slice(e * 128, (e + 1) * 128)
                nc.gpsimd.dma_start(out=xr_t[1][e][:], in_=d_xr.ap()[1, sl, :, :])
            for e in range(4):
                sl = slice(e * 128, (e + 1) * 128)
                nc.gpsimd.dma_start(out=xi_t[1][e][:], in_=d_xi.ap()[1, sl, :, :])
            # v-weights (r first), bd, rest of weights, smalls on sync
            for e in range(4):
                sl = slice(e * 128, (e + 1) * 128)
                nc.sync.dma_start(out=w_r[e][:, VS], in_=d_wr.ap()[sl, VS])
            for u in range(4):
                sl = slice(u * 128, (u + 1) * 128)
                nc.sync.dma_start(out=bd_r[u], in_=d_bdr.ap()[sl, :])
                nc.sync.dma_start(out=bd_in[u], in_=d_bdin.ap()[sl, :])
                nc.sync.dma_start(out=bd_i[u], in_=d_bdi.ap()[sl, :])
            nc.sync.dma_start(out=lg0, in_=d_lg0.ap())
            for d_w, w_t in ((d_wi, w_i), (d_ws, w_s)):
                for e in range(4):
                    sl = slice(e * 128, (e + 1) * 128)
                    nc.sync.dma_start(out=w_t[e][:, VS], in_=d_w.ap()[sl, VS])
            nc.sync.dma_start(out=v02, in_=d_v02.ap())
            nc.sync.dma_start(out=v02s, in_=d_v02s.ap())
            nc.sync.dma_start(out=id32, in_=d_id32.ap())
            nc.sync.dma_start(out=mask8, in_=d_mask.ap())
            nc.sync.dma_start(out=sel32, in_=d_sel.ap())
            for e in range(4):
                sl = slice(e * 128, (e + 1) * 128)
                nc.sync.dma_start(out=wcr[e], in_=d_wcr.ap()[sl, :])
                nc.sync.dma_start(out=wci[e], in_=d_wci.ap()[sl, :])


            stL = contextlib.ExitStack()
            psL = stL.enter_context(
                tc.tile_pool(name="psL", bufs=1, space="PSUM"))
            st = contextlib.ExitStack()
            psK = st.enter_context(
                tc.tile_pool(name="psK", bufs=1, space="PSUM"))

            nt = 0

            DR = mybir.MatmulPerfMode.DoubleRow

            def k_uhalf(p, uh):
                us = (2 * uh, 2 * uh + 1)
                for u in us:
                    fs = slice(u * 128, (u + 1) * 128)
                    tre = psK.tile([128, 512], F32, tag="t1", bufs=3,
                                   name=f"kre_{p}_{u}")
                    tim = psK.tile([128, 512], F32, tag="t2", bufs=3,
                                   name=f"kim_{p}_{u}")
                    for j, (wd, xd) in enumerate((("r", "r"), ("n", "i"))):
                        for c in range(2):
                            nc.tensor.matmul(tre[:], w8[wd][c][:, :, fs],
                                             x8[xd][p][c][:],
                                             start=(j == 0 and c == 0),
                                             stop=(j == 1 and c == 1),
                                             perf_mode=DR)
                    for j, (wd, xd) in enumerate((("i", "r"), ("r", "i"))):
                        for c in range(2):
                            nc.tensor.matmul(tim[:], w8[wd][c][:, :, fs],
                                             x8[xd][p][c][:],
                                             start=(j == 0 and c == 0),
                                             stop=(j == 1 and c == 1),
                                             perf_mode=DR)
                    if u % 2 == 0:
                        nc.scalar.copy(kTr[p][u][:], tre[:])
                        nc.vector.tensor_copy(kTi[p][u][:], tim[:])
                    else:
                        nc.vector.tensor_copy(kTr[p][u][:], tre[:])
                        nc.scalar.copy(kTi[p][u][:], tim[:])

            def logits_batch(b):
                p, hf = divmod(b, 2)
                cs = slice(hf * 256, (hf + 1) * 256)
                bs = slice(b * 8, (b + 1) * 8)
                lr = psL.tile([8, 256], F32, tag="lr", name=f"lr{b}")
                li = psL.tile([8, 256], F32, tag="li", name=f"li{b}")
                for u in range(4):
                    nc.tensor.matmul(lr[:], bd_r[u][:, bs], kTr[p][u][:, cs],
                                     start=(u == 0), stop=False)
                    nc.tensor.matmul(lr[:], bd_in[u][:, bs], kTi[p][u][:, cs],
                                     start=False, stop=(u == 3))
                for u in range(4):
                    nc.tensor.matmul(li[:], bd_r[u][:, bs], kTi[p][u][:, cs],
                                     start=(u == 0), stop=False)
                    nc.tensor.matmul(li[:], bd_i[u][:, bs], kTr[p][u][:, cs],
                                     start=False, stop=(u == 3))
                nc.scalar.activation(wexp[b][:, 0, :], lr[:], ACTF.Exp,
                                     bias=0.0, scale=1.0,
                                     accum_out=den8[b][:, 0:1])
                nc.scalar.activation(wexp[b][:, 1, :], li[:], ACTF.Exp,
                                     bias=0.0, scale=1.0,
                                     accum_out=den8[b][:, 1:2])
                # engines can't write at partition offset b*8; DMA can
                nc.scalar.dma_start(out=w_sm[bs, :, 1:S], in_=wexp[b][:])
                nc.scalar.dma_start(out=den[bs, :], in_=den8[b][:])

            # ---- k + logits, interleaved so PE never waits on combines ----
            k_uhalf(0, 0)
            k_uhalf(0, 1)
            k_uhalf(1, 0)
            logits_batch(0)
            logits_batch(1)
            k_uhalf(1, 1)
            for p in range(NPAIR):
                for e in range(4):
                    nc.vector.tensor_add(xs_t[p][e][:], xr_t[p][e][:],
                                         xi_t[p][e][:])
            logits_batch(2)
            logits_batch(3)
            st.close()   # psK

            # ---- softmax tail: s=0 column + normalization ----
            nc.scalar.activation(e0[:], lg0[:], ACTF.Exp, bias=0.0, scale=1.0)
            nc.vector.tensor_copy(w_sm[:, :, 0], e0[:])
            nc.vector.tensor_add(den2[:], den[:], e0[:])
            nc.vector.reciprocal(rs[:], den2[:])
            nc.vector.tensor_scalar_mul(w_sm[:, 0, :], w_sm[:, 0, :], rs[:, 0:1])
            nc.vector.tensor_scalar_mul(w_sm[:, 1, :], w_sm[:, 1, :], rs[:, 1:2])
            stL.close()  # psL

            stV = contextlib.ExitStack()
            psV = stV.enter_context(
                tc.tile_pool(name="psV", bufs=1, space="PSUM"))

            def v_batch(b):
                nonlocal nt
                p, hf = divmod(b, 2)
                for stt in range(2):
                    scs = slice(hf * 256 + stt * 128,
                                hf * 256 + (stt + 1) * 128)
                    tl = {}
                    for kind, bufs, w_off, x_k in (
                            ("t1", 2, w_r, xr_t), ("t2", 2, w_i, xi_t),
                            ("t3", 2, w_s, xs_t)):
                        t = psV.tile([128, 512], F32, tag=kind, bufs=bufs,
                                     name=f"v{kind}_{b}_{stt}")
                        tl[kind] = t
                        for e in range(4):
                            nc.tensor.matmul(t[:], x_k[p][e][:, scs],
                                             w_off[e][:, VS],
                                             start=(e == 0), stop=(e == 3))
                    # offload the SBUF-only part of this combine to gpsimd
                    c1, c2, sc = c1s[nt % 2], c2s[nt % 2], s12[nt % 2]
                    nc.scalar.copy(c1[:], tl["t1"][:])
                    nc.scalar.copy(c2[:], tl["t2"][:])
                    nc.gpsimd.tensor_sub(vr[b][stt][:], c1[:], c2[:])
                    nc.gpsimd.tensor_add(sc[:], c1[:], c2[:])
                    nc.vector.tensor_sub(vi[b][stt][:], tl["t3"][:], sc[:])
                    nt += 1

            def hv_batch(b, psH):
                bs = slice(b * 8, (b + 1) * 8)
                ph_r = psH.tile([NH, OUT], F32, tag="hr", name=f"phr{b}")
                ph_i = psH.tile([NH, OUT], F32, tag="hi", name=f"phi{b}")
                mm = nc.tensor.matmul
                mm(ph_r[:], wTr[0][:, bs], vr[b][0][:], start=True, stop=False)
                mm(ph_r[:], wTr[1][:, bs], vr[b][1][:], start=False, stop=False)
                mm(ph_r[:], wTin[0][:, bs], vi[b][0][:], start=False, stop=False)
                mm(ph_r[:], wTin[1][:, bs], vi[b][1][:], start=False, stop=False)
                mm(ph_r[:], wt0a[:, bs], v02[:, b, :], start=False, stop=True)
                mm(ph_i[:], wTi[0][:, bs], vr[b][0][:], start=True, stop=False)
                mm(ph_i[:], wTi[1][:, bs], vr[b][1][:], start=False, stop=False)
                mm(ph_i[:], wTr[0][:, bs], vi[b][0][:], start=False, stop=False)
                mm(ph_i[:], wTr[1][:, bs], vi[b][1][:], start=False, stop=False)
                mm(ph_i[:], wt0b[:, bs], v02s[:, b, :], start=False, stop=True)
                nc.vector.tensor_mul(hvm_rb[b][:], ph_r[:], mask8[:])
                nc.vector.tensor_mul(hvm_ib[b][:], ph_i[:], mask8[:])
                nc.gpsimd.dma_start(out=hvm_r[bs, :], in_=hvm_rb[b][:])
                nc.gpsimd.dma_start(out=hvm_i[bs, :], in_=hvm_ib[b][:])

            v_batch(0)

            # ---- softmax-weight transposes -> [128s, 32bh] ----
            with tc.tile_pool(name="psT", bufs=1, space="PSUM") as psT:
                for a in range(2):
                    cs = slice(1 + a * 128, 1 + (a + 1) * 128)
                    ptr = psT.tile([128, 32], BF, tag="tw", bufs=2, name=f"ptr{a}")
                    pti = psT.tile([128, 32], BF, tag="tw", bufs=2, name=f"pti{a}")
                    nc.tensor.transpose(ptr[:], w_sm[:, 0, cs], id32[:])
                    nc.tensor.transpose(pti[:], w_sm[:, 1, cs], id32[:])
                    nc.scalar.copy(wTr[a][:], ptr[:])
                    nc.scalar.copy(wTi[a][:], pti[:])
                    nc.scalar.activation(wTin[a][:], pti[:], ACTF.Copy,
                                         bias=0.0, scale=-1.0)
                # s=0 row of both parts in one [32, 2] -> [2, 32] transpose
                ptc_t = psT.tile([128, 32], BF, tag="tw", bufs=2, name="ptc")
                ptc = ptc_t[0:2, :]
                nc.tensor.transpose(ptc[:], w_sm[:, :, 0], id32[:])
                nc.scalar.copy(wt0b[:], ptc[:])
                nc.scalar.activation(wt0n[:], ptc[:], ACTF.Copy,
                                     bias=0.0, scale=-1.0)
                nc.sync.dma_start(out=wt0a[0:1, :], in_=wt0b[0:1, :])
                nc.sync.dma_start(out=wt0a[1:2, :], in_=wt0n[1:2, :])

            stH = contextlib.ExitStack()
            psH = stH.enter_context(
                tc.tile_pool(name="psH", bufs=1, space="PSUM"))
            v_batch(1)
            hv_batch(0, psH)
            v_batch(2)
            hv_batch(1, psH)
            v_batch(3)
            hv_batch(2, psH)
            hv_batch(3, psH)
            stH.close()  # psH
            stV.close()  # psV

            with tc.tile_pool(name="psY", bufs=1, space="PSUM") as psY:
                # ---- extract attn0^T [128f, 4b] via selection matmul ----
                for u in range(4):
                    fs = slice(u * 128, (u + 1) * 128)
                    par = psY.tile([128, BPC], F32, tag="par", bufs=2, name=f"par{u}")
                    pai = psY.tile([128, BPC], F32, tag="pai", bufs=2, name=f"pai{u}")
                    nc.tensor.matmul(par[:], hvm_r[:, fs], sel32[:],
                                     start=True, stop=True)
                    nc.tensor.matmul(pai[:], hvm_i[:, fs], sel32[:],
                                     start=True, stop=True)
                    nc.scalar.copy(att_r[u][:], par[:])
                    nc.scalar.copy(att_i[u][:], pai[:])
                    nc.scalar.activation(att_in[u][:], pai[:], ACTF.Copy,
                                         bias=0.0, scale=-1.0)

                # ---- y = attn0 @ Wc^T ----
                py_r = psY.tile([BPC, OUT], F32, tag="pyr")
                py_i = psY.tile([BPC, OUT], F32, tag="pyi")
                for j, u in enumerate(range(4)):
                    nc.tensor.matmul(py_r[:], att_r[u][:], wcr[u][:],
                                     start=(j == 0), stop=False)
                    nc.tensor.matmul(py_r[:], att_in[u][:], wci[u][:],
                                     start=False, stop=(j == 3))
                    nc.tensor.matmul(py_i[:], att_r[u][:], wci[u][:],
                                     start=(j == 0), stop=False)
                    nc.tensor.matmul(py_i[:], att_i[u][:], wcr[u][:],
                                     start=False, stop=(j == 3))
                nc.scalar.copy(y_r[:], py_r[:])
                nc.vector.tensor_copy(y_i[:], py_i[:])
                nc.sync.dma_start(out=d_yr.ap(), in_=y_r[:])
                nc.scalar.dma_start(out=d_yi.ap(), in_=y_i[:])

    nc.compile()
    return nc


def _host_prep(inputs):
    """Host-side math + per-core in_maps."""
    f32 = np.float32
    xr = np.ascontiguousarray(inputs["x_real"], dtype=f32).reshape(B, E, HW)
    xi = np.ascontiguousarray(inputs["x_imag"], dtype=f32).reshape(B, E, HW)
    pos = np.asarray(inputs["pos_r"], f32) + 1j * np.asarray(inputs["pos_i"], f32)
    w_in = np.asarray(inputs["w_in_r"], f32) + 1j * np.asarray(inputs["w_in_i"], f32)
    b_in = np.asarray(inputs["b_in_r"], f32) + 1j * np.asarray(inputs["b_in_i"], f32)
    w_out = np.asarray(inputs["w_out_r"], f32) + 1j * np.asarray(inputs["w_out_i"], f32)
    b_out = np.asarray(inputs["b_out_r"], f32) + 1j * np.asarray(inputs["b_out_i"], f32)
    w_p = np.asarray(inputs["w_p_r"], f32) + 1j * np.asarray(inputs["w_p_i"], f32)
    b_p = np.asarray(inputs["b_p_r"], f32) + 1j * np.asarray(inputs["b_p_i"], f32)

    # ---- host math for the s=0 (mean) token ----
    x0 = (xr.mean(-1, dtype=np.float64) + 1j * xi.mean(-1, dtype=np.float64)
          ).astype(np.complex64) + pos[:, 0]                     # [B, E]
    qs = 1.0 / np.sqrt(HD)
    q0 = (x0 @ w_in[:E].T + b_in[:E]) * qs                       # [B, E]
    k0 = x0 @ w_in[E:2 * E].T                                    # [B, E]
    v0 = x0 @ w_in[2 * E:].T                                     # [B, E]
    lg0c = np.einsum("bhd,bhd->bh", q0.reshape(B, NH, HD),
                     k0.reshape(B, NH, HD))                      # [B, NH]

    wc = w_p @ w_out                                             # [OUT, E]
    # v-bias exits via sum(softmax)=1; out/proj biases are linear offsets.
    b_v = b_in[2 * E:]
    y_bias = ((1 + 1j) * b_v) @ wc.T + b_out @ w_p.T + b_p       # [OUT]

    # pos folded into the shipped x; pre-paired [pair, E, 2b, 256s]
    xr_f = (xr + pos.real[None, :, 1:S]).reshape(NCORES, NPAIR, 2, E, HW)
    xi_f = (xi + pos.imag[None, :, 1:S]).reshape(NCORES, NPAIR, 2, E, HW)
    xr_f = np.ascontiguousarray(xr_f.transpose(0, 1, 3, 2, 4))
    xi_f = np.ascontiguousarray(xi_f.transpose(0, 1, 3, 2, 4))  # [c,p,E,2,HW]
    xr_s = xr_f.astype(BF16)
    xi_s = xi_f.astype(BF16)
    # fp8 k-path copies: [core, pair, chunk-pair, 128, half, (2b x 256s)]
    FP8 = ml_dtypes.float8_e4m3

    def to8(xf):
        a = (xf * 8.0).reshape(NCORES, NPAIR, 2, 2, 128, 2 * HW)
        return np.ascontiguousarray(a.transpose(0, 1, 2, 4, 3, 5)).astype(FP8)

    x8r = to8(xr_f)
    x8i = to8(xi_f)

    bf = lambda a: np.ascontiguousarray(a, dtype=f32).astype(BF16)
    wkv = w_in[E:].T                                             # [E, 2E] complex

    def w_to8(wk):
        a = np.ascontiguousarray(wk * 32.0).reshape(2, 2, 128, 512)
        return np.ascontiguousarray(a.transpose(0, 2, 1, 3)).astype(FP8)

    wk_r, wk_i = wkv.real[:, :E], wkv.imag[:, :E]
    shared = dict(
        wr=bf(wkv.real), wi=bf(wkv.imag), ws=bf(wkv.real + wkv.imag),
        wcr=bf(wc.real.T), wci=bf(wc.imag.T),
        w8r=w_to8(wk_r), w8i=w_to8(wk_i), w8n=w_to8(-wk_i),
        id32=np.eye(32, dtype=f32).astype(BF16),
    )
    mask8 = np.zeros((NH, OUT), f32)
    for h in range(NH):
        mask8[h, h * HD:(h + 1) * HD] = 1.0
    sel32 = np.zeros((32, BPC), f32)
    for b in range(BPC):
        sel32[b * 8:(b + 1) * 8, b] = 1.0
    shared["mask8"] = mask8.astype(BF16)
    shared["sel32"] = sel32.astype(BF16)

    in_maps = []
    for c in range(NCORES):
        bsl = slice(c * BPC, (c + 1) * BPC)
        q0c, v0c, lg0c_c = q0[bsl], v0[bsl], lg0c[bsl]
        # block-diag bd [E, 32]: row f (grouped per u-tile), col b*8 + h(f)
        bdr = np.zeros((E, 32), f32)
        bdi = np.zeros((E, 32), f32)
        fidx = np.arange(E)
        for b in range(BPC):
            bdr[fidx, b * 8 + fidx // HD] = q0c[b].real / 256.0
            bdi[fidx, b * 8 + fidx // HD] = q0c[b].imag / 256.0
        lg0m = np.empty((32, 2), f32)
        lg0m[:, 0] = lg0c_c.real.reshape(-1)
        lg0m[:, 1] = lg0c_c.imag.reshape(-1)
        v02 = np.stack([v0c.real.astype(f32), v0c.imag.astype(f32)])  # [2,BPC,OUT]
        m = dict(shared)
        m["bdr"] = bdr.astype(BF16)
        m["bdi"] = bdi.astype(BF16)
        m["bdin"] = (-bdi).astype(BF16)
        m["lg0"] = lg0m
        m["v02"] = v02.astype(BF16)
        m["v02s"] = v02[::-1].copy().astype(BF16)
        m["xr"] = xr_s[c]
        m["xi"] = xi_s[c]
        m["x8r"] = x8r[c]
        m["x8i"] = x8i[c]
        in_maps.append(m)
    return in_maps, y_bias.astype(np.complex64)


def _run(inputs, trace=False, **kw):
    from concourse.bass_utils import run_bass_kernel_spmd
    if "nc" not in _cached:
        _cached["nc"] = _build()
    nc = _cached["nc"]
    in_maps, y_bias = _host_prep(inputs)
    res = run_bass_kernel_spmd(nc, in_maps, core_ids=list(range(NCORES)),
                               trace=trace, **kw)
    out = np.empty((B, OUT), np.complex64)
    for c in range(NCORES):
        out[c * BPC:(c + 1) * BPC] = (res.results[c]["yr"]
                                      + 1j * res.results[c]["yi"])
    out += y_bias[None, :]
    return out, res


def kernel(**inputs) -> np.ndarray:
    out, _ = _run(inputs)
    return out


# revision 25
# speedup vs baseline: 1.0437x; 1.0437x over previous
"""Complex AttentionPool2d on 8 trn2 NeuronCores, data-parallel over batch.

Contract: kernel(**inputs) takes the FULL inputs from setup_inputs() and
returns the FULL [32, 512] complex64 output.

v3 design: all matmuls in bf16 (tolerance is 2e-2), Karatsuba (3 real
matmuls) for the dominant k/v projections, and every small/serial piece of
math moved to the host:
  host: pos folded into the shipped x (x' = pixels + pos, pre-paired
        [pair, E, 2b, 256s]); x0 = mean(x)+pos0; q0 = (x0 Wq + b_q)/8;
        k0 = x0 Wk; v0 = x0 Wv; logit[s=0] = q0.k0; block-diag bd tiles
        from q0; v0 row-pairs; y-bias added to the final output on host.
  device (per core, 4 batches as 2 column-packed pairs):
        kT[f, (b,s)]   = Wk @ x'                 # Karatsuba, f-major
        v[(b,st)]      = x'^T @ Wv               # Karatsuba, s-major
        logits[8h,256] = bd^T @ kT per batch; col 0 from host
        w = softmax(re) + i softmax(im)          # exp straight from PSUM
        attn0 = w^T v (+ w0 x v0 row term)       # per batch [8, 512]
        y = attn0 @ (w_p w_out)^T                # via sel-extracted att^T

Engine rules honored: GPSIMD can't touch PSUM; vector ops read at most one
PSUM operand (stage via scalar-engine copies); compute engines can't write
at non-32-aligned partition offsets (assemble via DMA).

Math identities: k-bias dropped (softmax invariant); v-bias exits through
sum(w)=1 as a constant y-offset (host-added); q-bias folded into host q0.
"""
import contextlib
import numpy as np
import ml_dtypes

B, E, HW, S = 32, 512, 256, 257
NH, HD = 8, 64
OUT = 512
NCORES = 8
BPC = B // NCORES   # batches per core
NPAIR = BPC // 2    # column-packed batch pairs
BF16 = ml_dtypes.bfloat16

_cached = {}


def _build():
    import concourse.bacc as bacc
    import concourse.tile as tile
    import concourse.mybir as mybir

    F32 = mybir.dt.float32
    BF = mybir.dt.bfloat16
    ACTF = mybir.ActivationFunctionType

    nc = bacc.Bacc("TRN2", target_bir_lowering=False, debug=False)

    # ---- DRAM I/O ----
    F8 = mybir.dt.float8e4
    d_xr = nc.dram_tensor("xr", [NPAIR, E, 2, HW], BF, kind="ExternalInput")
    d_xi = nc.dram_tensor("xi", [NPAIR, E, 2, HW], BF, kind="ExternalInput")
    d_x8r = nc.dram_tensor("x8r", [NPAIR, 2, 128, 2, 512], F8, kind="ExternalInput")
    d_x8i = nc.dram_tensor("x8i", [NPAIR, 2, 128, 2, 512], F8, kind="ExternalInput")
    d_w8r = nc.dram_tensor("w8r", [2, 128, 2, 512], F8, kind="ExternalInput")
    d_w8i = nc.dram_tensor("w8i", [2, 128, 2, 512], F8, kind="ExternalInput")
    d_w8n = nc.dram_tensor("w8n", [2, 128, 2, 512], F8, kind="ExternalInput")
    d_wr = nc.dram_tensor("wr", [E, 2 * E], BF, kind="ExternalInput")
    d_wi = nc.dram_tensor("wi", [E, 2 * E], BF, kind="ExternalInput")
    d_ws = nc.dram_tensor("ws", [E, 2 * E], BF, kind="ExternalInput")
    d_wcr = nc.dram_tensor("wcr", [E, OUT], BF, kind="ExternalInput")
    d_wci = nc.dram_tensor("wci", [E, OUT], BF, kind="ExternalInput")
    d_bdr = nc.dram_tensor("bdr", [2, 128, 2, 32], F8, kind="ExternalInput")
    d_bdi = nc.dram_tensor("bdi", [2, 128, 2, 32], F8, kind="ExternalInput")
    d_bdin = nc.dram_tensor("bdin", [2, 128, 2, 32], F8, kind="ExternalInput")
    d_lg0 = nc.dram_tensor("lg0", [32, 2], F32, kind="ExternalInput")
    d_v02 = nc.dram_tensor("v02", [2, BPC, OUT], BF, kind="ExternalInput")
    d_v02s = nc.dram_tensor("v02s", [2, BPC, OUT], BF, kind="ExternalInput")
    d_id32 = nc.dram_tensor("id32", [32, 32], BF, kind="ExternalInput")
    d_mask = nc.dram_tensor("mask8", [NH, OUT], BF, kind="ExternalInput")
    d_sel = nc.dram_tensor("sel32", [32, BPC], BF, kind="ExternalInput")
    d_yr = nc.dram_tensor("yr", [BPC, OUT], F32, kind="ExternalOutput")
    d_yi = nc.dram_tensor("yi", [BPC, OUT], F32, kind="ExternalOutput")

    KS = slice(0, 512)       # k columns of the packed kv weight
    VS = slice(512, 1024)    # v columns

    with tile.TileContext(nc) as tc, \
         nc.allow_low_precision(reason="bf16 kernel; tolerance is 2e-2"):
        with tc.tile_pool(name="consts", bufs=1) as consts, \
             tc.tile_pool(name="keep", bufs=1) as keep:
            # ---- persistent weights / constants ----
            w_r = [consts.tile([128, 2 * E], BF, name=f"wr{e}") for e in range(4)]
            w_i = [consts.tile([128, 2 * E], BF, name=f"wi{e}") for e in range(4)]
            w_s = [consts.tile([128, 2 * E], BF, name=f"ws{e}") for e in range(4)]
            wcr = [consts.tile([128, OUT], BF, name=f"wcr{e}") for e in range(4)]
            wci = [consts.tile([128, OUT], BF, name=f"wci{e}") for e in range(4)]
            bd_r = [consts.tile([128, 2, 32], F8, name=f"bdr{u}") for u in range(2)]
            bd_i = [consts.tile([128, 2, 32], F8, name=f"bdi{u}") for u in range(2)]
            bd_in = [consts.tile([128, 2, 32], F8, name=f"bdin{u}") for u in range(2)]
            lg0 = consts.tile([32, 2], F32)
            v02 = consts.tile([2, BPC, OUT], BF)
            v02s = consts.tile([2, BPC, OUT], BF)
            id32 = consts.tile([32, 32], BF)
            mask8 = consts.tile([NH, OUT], BF)
            sel32 = consts.tile([32, BPC], BF)

            x8 = {dt: [[consts.tile([128, 2, 512], F8, name=f"x8{dt}_{p}_{c}")
                        for c in range(2)] for p in range(NPAIR)]
                  for dt in "ri"}
            w8 = {dt: [consts.tile([128, 2, 512], F8, name=f"w8{dt}_{c}")
                       for c in range(2)] for dt in "rin"}

            # ---- persistent activations ----
            xr_t = [[keep.tile([128, 512], BF, name=f"xr{p}_{e}")
                     for e in range(4)] for p in range(NPAIR)]
            xi_t = [[keep.tile([128, 512], BF, name=f"xi{p}_{e}")
                     for e in range(4)] for p in range(NPAIR)]
            xs_t = [[keep.tile([128, 512], BF, name=f"xs{p}_{e}")
                     for e in range(4)] for p in range(NPAIR)]
            kTr = [[keep.tile([128, 2, 512], F8, name=f"kTr{p}_{u}")
                    for u in range(2)] for p in range(NPAIR)]
            kTi = [[keep.tile([128, 2, 512], F8, name=f"kTi{p}_{u}")
                    for u in range(2)] for p in range(NPAIR)]
            vr = [[keep.tile([128, OUT], BF, name=f"vr{b}_{s}")
                   for s in range(2)] for b in range(BPC)]
            vi = [[keep.tile([128, OUT], BF, name=f"vi{b}_{s}")
                   for s in range(2)] for b in range(BPC)]
            w_sm = keep.tile([32, 2, S], BF)        # softmax weights (re|im)
            wexp = [keep.tile([NH, 2, 256], BF, name=f"wexp{b}")
                    for b in range(BPC)]            # per-batch exp staging
            den8 = [keep.tile([NH, 2], F32, name=f"den8_{b}") for b in range(BPC)]
            den = keep.tile([32, 2], F32)           # exp row-sums of cols 1..256
            e0 = keep.tile([32, 2], F32)            # exp of the s=0 logit
            den2 = keep.tile([32, 2], F32)
            rs = keep.tile([32, 2], F32)
            wTr = [keep.tile([128, 32], BF, name=f"wTr{a}") for a in range(2)]
            wTi = [keep.tile([128, 32], BF, name=f"wTi{a}") for a in range(2)]
            wTin = [keep.tile([128, 32], BF, name=f"wTin{a}") for a in range(2)]
            wt0a = keep.tile([2, 32], BF)           # rows (w0r, -w0i)
            wt0b = keep.tile([2, 32], BF)           # rows (w0r, w0i)
            wt0n = keep.tile([2, 32], BF)
            hvm_r = keep.tile([32, OUT], BF)
            hvm_i = keep.tile([32, OUT], BF)
            hvm_rb = [keep.tile([NH, OUT], BF, name=f"hvr{b}") for b in range(BPC)]
            hvm_ib = [keep.tile([NH, OUT], BF, name=f"hvi{b}") for b in range(BPC)]
            att_r = [keep.tile([128, BPC], BF, name=f"atr{u}") for u in range(4)]
            att_i = [keep.tile([128, BPC], BF, name=f"ati{u}") for u in range(4)]
            att_in = [keep.tile([128, BPC], BF, name=f"atn{u}") for u in range(4)]
            s12 = [keep.tile([128, 512], F32, name=f"s12_{j}") for j in range(2)]
            c2s = [keep.tile([128, 512], F32, name=f"c2_{j}") for j in range(2)]
            c1s = [keep.tile([128, 512], F32, name=f"c1_{j}") for j in range(2)]
            y_r = keep.tile([BPC, OUT], F32)
            y_i = keep.tile([BPC, OUT], F32)

            # ---- DMA emission; first-needed bytes first ----
            # fp8 k-path: weights on sync/scalar, x8 on gpsimd
            for c in range(2):
                nc.sync.dma_start(out=w8["r"][c], in_=d_w8r.ap()[c])
                nc.scalar.dma_start(out=w8["i"][c], in_=d_w8i.ap()[c])
                nc.sync.dma_start(out=w8["n"][c], in_=d_w8n.ap()[c])
            for c in range(2):
                nc.gpsimd.dma_start(out=x8["r"][0][c], in_=d_x8r.ap()[0, c])
                nc.gpsimd.dma_start(out=x8["i"][0][c], in_=d_x8i.ap()[0, c])
            for e in range(4):
                sl = slice(e * 128, (e + 1) * 128)
                nc.gpsimd.dma_start(out=xr_t[0][e][:], in_=d_xr.ap()[0, sl, :, :])
            for c in range(2):
                nc.gpsimd.dma_start(out=x8["r"][1][c], in_=d_x8r.ap()[1, c])
                nc.gpsimd.dma_start(out=x8["i"][1][c], in_=d_x8i.ap()[1, c])
            for e in range(4):
                sl = slice(e * 128, (e + 1) * 128)
                nc.gpsimd.dma_start(out=xi_t[0][e][:], in_=d_xi.ap()[0, sl, :, :])
            for e in range(4):
                sl = slice(e * 128, (e + 1) * 128)
                nc.gpsimd.dma_start(out=xr_t[1][e][:], in_=d_xr.ap()[1, sl, :, :])
            for e in range(4):
                sl = slice(e * 128, (e + 1) * 128)
                nc.gpsimd.dma_start(out=xi_t[1][e][:], in_=d_xi.ap()[1, sl, :, :])
            # v-weights (r first), bd, rest of weights, smalls on sync
            for e in range(4):
                sl = slice(e * 128, (e + 1) * 128)
                nc.sync.dma_start(out=w_r[e][:, VS], in_=d_wr.ap()[sl, VS])
            for u in range(2):
                nc.sync.dma_start(out=bd_r[u], in_=d_bdr.ap()[u])
                nc.sync.dma_start(out=bd_in[u], in_=d_bdin.ap()[u])
                nc.sync.dma_start(out=bd_i[u], in_=d_bdi.ap()[u])
            nc.sync.dma_start(out=lg0, in_=d_lg0.ap())
            for d_w, w_t in ((d_wi, w_i), (d_ws, w_s)):
                for e in range(4):
                    sl = slice(e * 128, (e + 1) * 128)
                    nc.sync.dma_start(out=w_t[e][:, VS], in_=d_w.ap()[sl, VS])
            nc.sync.dma_start(out=v02, in_=d_v02.ap())
            nc.sync.dma_start(out=v02s, in_=d_v02s.ap())
            nc.sync.dma_start(out=id32, in_=d_id32.ap())
            nc.sync.dma_start(out=mask8, in_=d_mask.ap())
            nc.sync.dma_start(out=sel32, in_=d_sel.ap())
            for e in range(4):
                sl = slice(e * 128, (e + 1) * 128)
                nc.sync.dma_start(out=wcr[e], in_=d_wcr.ap()[sl, :])
                nc.sync.dma_start(out=wci[e], in_=d_wci.ap()[sl, :])


            # PE p-state warm-up: ~3us of dummy matmuls while DMAs land
            dummy = keep.tile([128, 512], BF)
            nc.gpsimd.memset(dummy[:], 0.0)
            with tc.tile_pool(name="psW", bufs=1, space="PSUM") as psW:
                pw = psW.tile([128, 128], F32, tag="w", bufs=2)
                for j in range(14):
                    nc.tensor.matmul(pw[:], dummy[:, 0:128], dummy[:, 0:128],
                                     start=True, stop=True)

            stL = contextlib.ExitStack()
            psL = stL.enter_context(
                tc.tile_pool(name="psL", bufs=1, space="PSUM"))
            st = contextlib.ExitStack()
            psK = st.enter_context(
                tc.tile_pool(name="psK", bufs=1, space="PSUM"))

            nt = 0

            DR = mybir.MatmulPerfMode.DoubleRow

            def k_uhalf(p, uh):
                us = (2 * uh, 2 * uh + 1)
                for u in us:
                    fs = slice(u * 128, (u + 1) * 128)
                    tre = psK.tile([128, 512], F32, tag="t1", bufs=3,
                                   name=f"kre_{p}_{u}")
                    tim = psK.tile([128, 512], F32, tag="t2", bufs=3,
                                   name=f"kim_{p}_{u}")
                    for j, (wd, xd) in enumerate((("r", "r"), ("n", "i"))):
                        for c in range(2):
                            nc.tensor.matmul(tre[:], w8[wd][c][:, :, fs],
                                             x8[xd][p][c][:],
                                             start=(j == 0 and c == 0),
                                             stop=(j == 1 and c == 1),
                                             perf_mode=DR)
                    for j, (wd, xd) in enumerate((("i", "r"), ("r", "i"))):
                        for c in range(2):
                            nc.tensor.matmul(tim[:], w8[wd][c][:, :, fs],
                                             x8[xd][p][c][:],
                                             start=(j == 0 and c == 0),
                                             stop=(j == 1 and c == 1),
                                             perf_mode=DR)
                    nc.scalar.activation(kTr[p][u // 2][:, u % 2, :], tre[:],
                                         ACTF.Copy, bias=0.0, scale=0.125)
                    nc.vector.tensor_scalar_mul(kTi[p][u // 2][:, u % 2, :],
                                                tim[:], 0.125)

            def logits_batch(b):
                p, hf = divmod(b, 2)
                cs = slice(hf * 256, (hf + 1) * 256)
                bs = slice(b * 8, (b + 1) * 8)
                lr = psL.tile([8, 256], F32, tag="lr", name=f"lr{b}")
                li = psL.tile([8, 256], F32, tag="li", name=f"li{b}")
                for u in range(2):
                    nc.tensor.matmul(lr[:], bd_r[u][:, :, bs],
                                     kTr[p][u][:, :, cs],
                                     start=(u == 0), stop=False, perf_mode=DR)
                    nc.tensor.matmul(lr[:], bd_in[u][:, :, bs],
                                     kTi[p][u][:, :, cs],
                                     start=False, stop=(u == 1), perf_mode=DR)
                for u in range(2):
                    nc.tensor.matmul(li[:], bd_r[u][:, :, bs],
                                     kTi[p][u][:, :, cs],
                                     start=(u == 0), stop=False, perf_mode=DR)
                    nc.tensor.matmul(li[:], bd_i[u][:, :, bs],
                                     kTr[p][u][:, :, cs],
                                     start=False, stop=(u == 1), perf_mode=DR)
                LS = 1.0 / 16384.0
                nc.scalar.activation(wexp[b][:, 0, :], lr[:], ACTF.Exp,
                                     bias=0.0, scale=LS,
                                     accum_out=den8[b][:, 0:1])
                nc.scalar.activation(wexp[b][:, 1, :], li[:], ACTF.Exp,
                                     bias=0.0, scale=LS,
                                     accum_out=den8[b][:, 1:2])
                # engines can't write at partition offset b*8; DMA can
                nc.scalar.dma_start(out=w_sm[bs, :, 1:S], in_=wexp[b][:])
                nc.scalar.dma_start(out=den[bs, :], in_=den8[b][:])

            # ---- k + logits, interleaved so PE never waits on combines ----
            k_uhalf(0, 0)
            k_uhalf(0, 1)
            k_uhalf(1, 0)
            logits_batch(0)
            logits_batch(1)
            k_uhalf(1, 1)
            for p in range(NPAIR):
                for e in range(4):
                    nc.vector.tensor_add(xs_t[p][e][:], xr_t[p][e][:],
                                         xi_t[p][e][:])
            logits_batch(2)
            logits_batch(3)
            st.close()   # psK

            # ---- softmax tail: s=0 column + normalization ----
            nc.scalar.activation(e0[:], lg0[:], ACTF.Exp, bias=0.0, scale=1.0)
            nc.vector.tensor_copy(w_sm[:, :, 0], e0[:])
            nc.vector.tensor_add(den2[:], den[:], e0[:])
            nc.vector.reciprocal(rs[:], den2[:])
            nc.vector.tensor_scalar_mul(w_sm[:, 0, :], w_sm[:, 0, :], rs[:, 0:1])
            nc.vector.tensor_scalar_mul(w_sm[:, 1, :], w_sm[:, 1, :], rs[:, 1:2])
            stL.close()  # psL

            stV = contextlib.ExitStack()
            psV = stV.enter_context(
                tc.tile_pool(name="psV", bufs=1, space="PSUM"))

            def v_batch(b):
                nonlocal nt
                p, hf = divmod(b, 2)
                for stt in range(2):
                    scs = slice(hf * 256 + stt * 128,
                                hf * 256 + (stt + 1) * 128)
                    tl = {}
                    for kind, bufs, w_off, x_k in (
                            ("t1", 2, w_r, xr_t), ("t2", 2, w_i, xi_t),
                            ("t3", 2, w_s, xs_t)):
                        t = psV.tile([128, 512], F32, tag=kind, bufs=bufs,
                                     name=f"v{kind}_{b}_{stt}")
                        tl[kind] = t
                        for e in range(4):
                            nc.tensor.matmul(t[:], x_k[p][e][:, scs],
                                             w_off[e][:, VS],
                                             start=(e == 0), stop=(e == 3))
                    # offload the SBUF-only part of this combine to gpsimd
                    c1, c2, sc = c1s[nt % 2], c2s[nt % 2], s12[nt % 2]
                    nc.scalar.copy(c1[:], tl["t1"][:])
                    nc.scalar.copy(c2[:], tl["t2"][:])
                    nc.gpsimd.tensor_sub(vr[b][stt][:], c1[:], c2[:])
                    nc.gpsimd.tensor_add(sc[:], c1[:], c2[:])
                    nc.vector.tensor_sub(vi[b][stt][:], tl["t3"][:], sc[:])
                    nt += 1

            def hv_batch(b, psH):
                bs = slice(b * 8, (b + 1) * 8)
                ph_r = psH.tile([NH, OUT], F32, tag="hr", name=f"phr{b}")
                ph_i = psH.tile([NH, OUT], F32, tag="hi", name=f"phi{b}")
                mm = nc.tensor.matmul
                mm(ph_r[:], wTr[0][:, bs], vr[b][0][:], start=True, stop=False)
                mm(ph_r[:], wTr[1][:, bs], vr[b][1][:], start=False, stop=False)
                mm(ph_r[:], wTin[0][:, bs], vi[b][0][:], start=False, stop=False)
                mm(ph_r[:], wTin[1][:, bs], vi[b][1][:], start=False, stop=False)
                mm(ph_r[:], wt0a[:, bs], v02[:, b, :], start=False, stop=True)
                mm(ph_i[:], wTi[0][:, bs], vr[b][0][:], start=True, stop=False)
                mm(ph_i[:], wTi[1][:, bs], vr[b][1][:], start=False, stop=False)
                mm(ph_i[:], wTr[0][:, bs], vi[b][0][:], start=False, stop=False)
                mm(ph_i[:], wTr[1][:, bs], vi[b][1][:], start=False, stop=False)
                mm(ph_i[:], wt0b[:, bs], v02s[:, b, :], start=False, stop=True)
                nc.vector.tensor_mul(hvm_rb[b][:], ph_r[:], mask8[:])
                nc.vector.tensor_mul(hvm_ib[b][:], ph_i[:], mask8[:])
                nc.gpsimd.dma_start(out=hvm_r[bs, :], in_=hvm_rb[b][:])
                nc.gpsimd.dma_start(out=hvm_i[bs, :], in_=hvm_ib[b][:])

            v_batch(0)

            # ---- softmax-weight transposes -> [128s, 32bh] ----
            with tc.tile_pool(name="psT", bufs=1, space="PSUM") as psT:
                for a in range(2):
                    cs = slice(1 + a * 128, 1 + (a + 1) * 128)
                    ptr = psT.tile([128, 32], BF, tag="tw", bufs=2, name=f"ptr{a}")
                    pti = psT.tile([128, 32], BF, tag="tw", bufs=2, name=f"pti{a}")
                    nc.tensor.transpose(ptr[:], w_sm[:, 0, cs], id32[:])
                    nc.tensor.transpose(pti[:], w_sm[:, 1, cs], id32[:])
                    nc.scalar.copy(wTr[a][:], ptr[:])
                    nc.scalar.copy(wTi[a][:], pti[:])
                    nc.scalar.activation(wTin[a][:], pti[:], ACTF.Copy,
                                         bias=0.0, scale=-1.0)
                # s=0 row of both parts in one [32, 2] -> [2, 32] transpose
                ptc_t = psT.tile([128, 32], BF, tag="tw", bufs=2, name="ptc")
                ptc = ptc_t[0:2, :]
                nc.tensor.transpose(ptc[:], w_sm[:, :, 0], id32[:])
                nc.scalar.copy(wt0b[:], ptc[:])
                nc.scalar.activation(wt0n[:], ptc[:], ACTF.Copy,
                                     bias=0.0, scale=-1.0)
                nc.sync.dma_start(out=wt0a[0:1, :], in_=wt0b[0:1, :])
                nc.sync.dma_start(out=wt0a[1:2, :], in_=wt0n[1:2, :])

            stH = contextlib.ExitStack()
            psH = stH.enter_context(
                tc.tile_pool(name="psH", bufs=1, space="PSUM"))
            v_batch(1)
            hv_batch(0, psH)
            v_batch(2)
            hv_batch(1, psH)
            v_batch(3)
            hv_batch(2, psH)
            hv_batch(3, psH)
            stH.close()  # psH
            stV.close()  # psV

            with tc.tile_pool(name="psY", bufs=1, space="PSUM") as psY:
                # ---- extract attn0^T [128f, 4b] via selection matmul ----
                for u in range(4):
                    fs = slice(u * 128, (u + 1) * 128)
                    par = psY.tile([128, BPC], F32, tag="par", bufs=2, name=f"par{u}")
                    pai = psY.tile([128, BPC], F32, tag="pai", bufs=2, name=f"pai{u}")
                    nc.tensor.matmul(par[:], hvm_r[:, fs], sel32[:],
                                     start=True, stop=True)
                    nc.tensor.matmul(pai[:], hvm_i[:, fs], sel32[:],
                                     start=True, stop=True)
                    nc.scalar.copy(att_r[u][:], par[:])
                    nc.scalar.copy(att_i[u][:], pai[:])
                    nc.scalar.activation(att_in[u][:], pai[:], ACTF.Copy,
                                         bias=0.0, scale=-1.0)

                # ---- y = attn0 @ Wc^T ----
                py_r = psY.tile([BPC, OUT], F32, tag="pyr")
                py_i = psY.tile([BPC, OUT], F32, tag="pyi")
                for j, u in enumerate(range(4)):
                    nc.tensor.matmul(py_r[:], att_r[u][:], wcr[u][:],
                                     start=(j == 0), stop=False)
                    nc.tensor.matmul(py_r[:], att_in[u][:], wci[u][:],
                                     start=False, stop=(j == 3))
                    nc.tensor.matmul(py_i[:], att_r[u][:], wci[u][:],
                                     start=(j == 0), stop=False)
                    nc.tensor.matmul(py_i[:], att_i[u][:], wcr[u][:],
                                     start=False, stop=(j == 3))
                nc.scalar.copy(y_r[:], py_r[:])
                nc.vector.tensor_copy(y_i[:], py_i[:])
                nc.sync.dma_start(out=d_yr.ap(), in_=y_r[:])
                nc.scalar.dma_start(out=d_yi.ap(), in_=y_i[:])

    nc.compile()
    return nc


def _host_prep(inputs):
    """Host-side math + per-core in_maps."""
    f32 = np.float32
    xr = np.ascontiguousarray(inputs["x_real"], dtype=f32).reshape(B, E, HW)
    xi = np.ascontiguousarray(inputs["x_imag"], dtype=f32).reshape(B, E, HW)
    pos = np.asarray(inputs["pos_r"], f32) + 1j * np.asarray(inputs["pos_i"], f32)
    w_in = np.asarray(inputs["w_in_r"], f32) + 1j * np.asarray(inputs["w_in_i"], f32)
    b_in = np.asarray(inputs["b_in_r"], f32) + 1j * np.asarray(inputs["b_in_i"], f32)
    w_out = np.asarray(inputs["w_out_r"], f32) + 1j * np.asarray(inputs["w_out_i"], f32)
    b_out = np.asarray(inputs["b_out_r"], f32) + 1j * np.asarray(inputs["b_out_i"], f32)
    w_p = np.asarray(inputs["w_p_r"], f32) + 1j * np.asarray(inputs["w_p_i"], f32)
    b_p = np.asarray(inputs["b_p_r"], f32) + 1j * np.asarray(inputs["b_p_i"], f32)

    # ---- host math for the s=0 (mean) token ----
    x0 = (xr.mean(-1, dtype=np.float64) + 1j * xi.mean(-1, dtype=np.float64)
          ).astype(np.complex64) + pos[:, 0]                     # [B, E]
    qs = 1.0 / np.sqrt(HD)
    q0 = (x0 @ w_in[:E].T + b_in[:E]) * qs                       # [B, E]
    k0 = x0 @ w_in[E:2 * E].T                                    # [B, E]
    v0 = x0 @ w_in[2 * E:].T                                     # [B, E]
    lg0c = np.einsum("bhd,bhd->bh", q0.reshape(B, NH, HD),
                     k0.reshape(B, NH, HD))                      # [B, NH]

    wc = w_p @ w_out                                             # [OUT, E]
    # v-bias exits via sum(softmax)=1; out/proj biases are linear offsets.
    b_v = b_in[2 * E:]
    y_bias = ((1 + 1j) * b_v) @ wc.T + b_out @ w_p.T + b_p       # [OUT]

    # pos folded into the shipped x; pre-paired [pair, E, 2b, 256s]
    xr_f = (xr + pos.real[None, :, 1:S]).reshape(NCORES, NPAIR, 2, E, HW)
    xi_f = (xi + pos.imag[None, :, 1:S]).reshape(NCORES, NPAIR, 2, E, HW)
    xr_f = np.ascontiguousarray(xr_f.transpose(0, 1, 3, 2, 4))
    xi_f = np.ascontiguousarray(xi_f.transpose(0, 1, 3, 2, 4))  # [c,p,E,2,HW]
    xr_s = xr_f.astype(BF16)
    xi_s = xi_f.astype(BF16)
    # fp8 k-path copies: [core, pair, chunk-pair, 128, half, (2b x 256s)]
    FP8 = ml_dtypes.float8_e4m3

    def to8(xf):
        a = (xf * 8.0).reshape(NCORES, NPAIR, 2, 2, 128, 2 * HW)
        return np.ascontiguousarray(a.transpose(0, 1, 2, 4, 3, 5)).astype(FP8)

    x8r = to8(xr_f)
    x8i = to8(xi_f)

    bf = lambda a: np.ascontiguousarray(a, dtype=f32).astype(BF16)
    wkv = w_in[E:].T                                             # [E, 2E] complex

    def w_to8(wk):
        a = np.ascontiguousarray(wk * 32.0).reshape(2, 2, 128, 512)
        return np.ascontiguousarray(a.transpose(0, 2, 1, 3)).astype(FP8)

    wk_r, wk_i = wkv.real[:, :E], wkv.imag[:, :E]
    shared = dict(
        wr=bf(wkv.real), wi=bf(wkv.imag), ws=bf(wkv.real + wkv.imag),
        wcr=bf(wc.real.T), wci=bf(wc.imag.T),
        w8r=w_to8(wk_r), w8i=w_to8(wk_i), w8n=w_to8(-wk_i),
        id32=np.eye(32, dtype=f32).astype(BF16),
    )
    mask8 = np.zeros((NH, OUT), f32)
    for h in range(NH):
        mask8[h, h * HD:(h + 1) * HD] = 1.0
    sel32 = np.zeros((32, BPC), f32)
    for b in range(BPC):
        sel32[b * 8:(b + 1) * 8, b] = 1.0
    shared["mask8"] = mask8.astype(BF16)
    shared["sel32"] = sel32.astype(BF16)

    in_maps = []
    for c in range(NCORES):
        bsl = slice(c * BPC, (c + 1) * BPC)
        q0c, v0c, lg0c_c = q0[bsl], v0[bsl], lg0c[bsl]
        # block-diag bd [E, 32]: row f (grouped per u-tile), col b*8 + h(f)
        bdr = np.zeros((E, 32), f32)
        bdi = np.zeros((E, 32), f32)
        fidx = np.arange(E)
        for b in range(BPC):
            bdr[fidx, b * 8 + fidx // HD] = q0c[b].real * 512.0
            bdi[fidx, b * 8 + fidx // HD] = q0c[b].imag * 512.0
        # DR pair layout [up, 128, half, 32]
        bdr = np.ascontiguousarray(
            bdr.reshape(2, 2, 128, 32).transpose(0, 2, 1, 3))
        bdi = np.ascontiguousarray(
            bdi.reshape(2, 2, 128, 32).transpose(0, 2, 1, 3))
        lg0m = np.empty((32, 2), f32)
        lg0m[:, 0] = lg0c_c.real.reshape(-1)
        lg0m[:, 1] = lg0c_c.imag.reshape(-1)
        v02 = np.stack([v0c.real.astype(f32), v0c.imag.astype(f32)])  # [2,BPC,OUT]
        m = dict(shared)
        m["bdr"] = bdr.astype(FP8)
        m["bdi"] = bdi.astype(FP8)
        m["bdin"] = (-bdi).astype(FP8)
        m["lg0"] = lg0m
        m["v02"] = v02.astype(BF16)
        m["v02s"] = v02[::-1].copy().astype(BF16)
        m["xr"] = xr_s[c]
        m["xi"] = xi_s[c]
        m["x8r"] = x8r[c]
        m["x8i"] = x8i[c]
        in_maps.append(m)
    return in_maps, y_bias.astype(np.complex64)


def _run(inputs, trace=False, **kw):
    from concourse.bass_utils import run_bass_kernel_spmd
    if "nc" not in _cached:
        _cached["nc"] = _build()
    nc = _cached["nc"]
    in_maps, y_bias = _host_prep(inputs)
    res = run_bass_kernel_spmd(nc, in_maps, core_ids=list(range(NCORES)),
                               trace=trace, **kw)
    out = np.empty((B, OUT), np.complex64)
    for c in range(NCORES):
        out[c * BPC:(c + 1) * BPC] = (res.results[c]["yr"]
                                      + 1j * res.results[c]["yi"])
    out += y_bias[None, :]
    return out, res


def kernel(**inputs) -> np.ndarray:
    out, _ = _run(inputs)
    return out
